# revision 29
# baseline (speedup 1.0000x reference)
"""Trainium2 Bass kernel for nn_CategoryMultiplier.

out[b, s, :] = inputs[b, s, :] * (emb_table[categories[b, s]] if
               categories[b, s] != 0 else 1.0)

Sharding: pure data parallel over batch. 8 cores x 16 batches each.
Precision: fp16 end-to-end (host converts f32->fp16 in, fp16->f32 out).

Gather strategy: NO gather at all. Every gather mechanism measured on
this stack bottlenecks (HBM dma_gather serializes ~8ns/row on the
single SWDGE queue = 72us; ap_gather ucode ~30ns/row; transpose-mode
gather crashes the runtime). Instead the embedding lookup runs as a
block-diagonal one-hot matmul on the otherwise-idle PE:

  The host partitions the vocab into 8 BINS of <=128 rows whose
  position counts are balanced to exactly N/8 = 1024 each (greedy +
  swap repair; falls back to contiguous bins + padding if an input
  can't be balanced). Positions are grouped by bin, so each 128-
  position block needs one matmul: stationary = one-hot.T [128v, 128p]
  (one DVE is_equal of the uint8 within-bin slot ids against a per-
  partition iota), moving = the bin's 128 table rows [128v, 512f]
  SBUF-resident, psum out = the gathered rows [128p, 512f]. ACT
  copies psum->fp16, DVE multiplies by x, GpSimd DMAs y out in sorted
  order, and the host inverse-permutes. Exact balance means zero x/y
  padding traffic.

Padding (category 0 -> multiplier 1.0): host writes ones into table
row 0; pad positions (fallback path only) carry slot id 255, which
matches no one-hot lane and yields y=0 (discarded).
"""

import numpy as np

import concourse.bacc as bacc
import concourse.mybir as mybir
import concourse.tile as tile
from concourse.bass_utils import run_bass_kernel_spmd

# Problem shape (hardcoded per harness contract).
B, S, D = 128, 512, 512
VOCAB = 1000
N_CORES = 8
B_LOC = B // N_CORES            # 16 batches per core
N = B_LOC * S                   # 8192 positions per core
P = 128                         # SBUF partitions
TILES = 8                       # vocab bins of <=128 rows
CAP_BAL = N // TILES            # 1024: per-bin positions when balanced

F16 = mybir.dt.float16
U8 = mybir.dt.uint8

_ALU = mybir.AluOpType


def _build_nc(cap):
    bpt = cap // P              # blocks per bin
    nblk = TILES * bpt
    sup = 4 if bpt % 4 == 0 else 3   # blocks per super-chunk
    assert bpt % sup == 0

    nc = bacc.Bacc("TRN2", target_bir_lowering=False, debug=False)

    xb = nc.dram_tensor("xb", [P, nblk * D], F16, kind="ExternalInput")
    catrel = nc.dram_tensor("catrel", [P, nblk * P], U8,
                            kind="ExternalInput")
    tabsb = nc.dram_tensor("tabsb", [P, TILES * D], F16,
                           kind="ExternalInput")
    yb = nc.dram_tensor("yb", [P, nblk * D], F16, kind="ExternalOutput")

    iota_dram = nc.inline_tensor(
        np.arange(P, dtype=np.float32).reshape(P, 1), name="iota_col")

    with tile.TileContext(nc) as tc:
        with (
            tc.tile_pool(name="const", bufs=1) as const_pool,
            tc.tile_pool(name="oh", bufs=TILES) as oh_pool,
            tc.tile_pool(name="io", bufs=6) as io_pool,
            tc.tile_pool(name="m", bufs=6) as m_pool,
            tc.psum_pool(name="ps", bufs=8 // sup) as ps_pool,
        ):
            iota_t = const_pool.tile([P, 1], mybir.dt.float32)
            nc.sync.dma_start(out=iota_t[:], in_=iota_dram[:])
            tab_t = const_pool.tile([P, TILES * D], F16)
            nc.sync.dma_start(out=tab_t[:], in_=tabsb[:])
            # cat slices per bin on the (otherwise idle-at-start) gpsimd
            # queue so bin 0's one-hot unblocks after ~1 small DMA.
            cat_ts = []
            for t in range(TILES):
                ct = const_pool.tile([P, cap], U8, tag=f"cat{t}")
                nc.gpsimd.dma_start(out=ct[:],
                                    in_=catrel[:, t * cap:(t + 1) * cap])
                cat_ts.append(ct)

            # All one-hots up front: DVE finishes them before the muls
            # queue up, so PE never stalls at a bin boundary.
            oh_ts = []
            for t in range(TILES):
                oh_t = oh_pool.tile([P, cap], F16, tag="oh")
                nc.vector.tensor_scalar(
                    out=oh_t[:], in0=cat_ts[t][:],
                    scalar1=iota_t[:, 0:1], scalar2=None, op0=_ALU.is_equal)
                oh_ts.append(oh_t)

            # Chunk schedule: sup-block chunks, except the last bin tapers
            # to a 1-block final chunk — the closing y DMA is then 0.5MB
            # instead of 2MB, cutting ~5us of post-compute drain tail.
            chunks = []
            for t in range(TILES):
                if t == TILES - 1 and sup == 4:
                    sizes = [4, 3, 1]
                else:
                    sizes = [sup] * (bpt // sup)
                off = 0
                for csz in sizes:
                    chunks.append((t, t * bpt + off, csz))
                    off += csz
            n_chunks = len(chunks)
            # Every 4th chunk (and the final one) drains PSUM via a direct
            # DVE multiply (f32 PSUM operand, 1x rate) instead of the ACT
            # copy, so the engines drain concurrently and PE never waits.
            dve_drain = set(range(2, n_chunks, 4)) | {n_chunks - 1}
            for ci, (t, blk0, csz) in enumerate(chunks):
                w = csz * D
                x_t = io_pool.tile([P, sup * D], F16, tag="x")
                nc.sync.dma_start(
                    out=x_t[:, :w], in_=xb[:, blk0 * D:blk0 * D + w])
                m_t = m_pool.tile([P, sup * D], F16, tag="m")
                ps_t = ps_pool.tile([P, sup * D], mybir.dt.float32)
                for j in range(csz):
                    lo = (blk0 - t * bpt + j) * P
                    nc.tensor.matmul(
                        ps_t[:, j * D:(j + 1) * D],
                        oh_ts[t][:, lo:lo + P],
                        tab_t[:, t * D:(t + 1) * D],
                        start=True, stop=True)
                if ci in dve_drain:
                    nc.vector.tensor_mul(out=m_t[:, :w], in0=x_t[:, :w],
                                         in1=ps_t[:, :w])
                else:
                    nc.scalar.copy(out=m_t[:, :w], in_=ps_t[:, :w])
                    nc.vector.tensor_mul(out=m_t[:, :w], in0=m_t[:, :w],
                                         in1=x_t[:, :w])
                nc.gpsimd.dma_start(
                    out=yb[:, blk0 * D:blk0 * D + w], in_=m_t[:, :w])

    nc.compile()
    return nc


_NC = {}


def _get_nc(cap=CAP_BAL):
    if cap not in _NC:
        _NC[cap] = _build_nc(cap)
    return _NC[cap]


def _balance_bins(counts):
    """Partition vocab rows into TILES bins of <=P rows with position
    counts summing exactly to CAP_BAL each. Returns row2bin or None."""
    order = np.argsort(-counts)
    bins = [[] for _ in range(TILES)]
    sums = np.zeros(TILES, np.int64)
    for v in order:
        for b in sorted(range(TILES), key=lambda b: (sums[b], len(bins[b]))):
            if len(bins[b]) < P:
                bins[b].append(int(v))
                sums[b] += counts[v]
                break
    for _ in range(20000):
        hi, lo = int(np.argmax(sums)), int(np.argmin(sums))
        if sums[hi] == CAP_BAL and sums[lo] == CAP_BAL:
            row2bin = np.empty(len(counts), np.int64)
            for b, rows in enumerate(bins):
                row2bin[rows] = b
            return row2bin
        diff = sums[hi] - CAP_BAL
        moved = False
        if len(bins[lo]) < P:
            best = None
            for v in bins[hi]:
                if 0 < counts[v] <= diff and (
                        best is None or counts[v] > counts[best]):
                    best = v
            if best is not None:
                bins[hi].remove(best)
                bins[lo].append(best)
                sums[hi] -= counts[best]
                sums[lo] += counts[best]
                moved = True
        if not moved:
            for a in bins[hi]:
                for bb in bins[lo]:
                    dd = counts[a] - counts[bb]
                    if 0 < dd <= diff:
                        bins[hi].remove(a)
                        bins[lo].remove(bb)
                        bins[hi].append(bb)
                        bins[lo].append(a)
                        sums[hi] -= dd
                        sums[lo] += dd
                        moved = True
                        break
                if moved:
                    break
        if not moved:
            return None
    return None


def _shard_inputs(inputs, categories, emb_table):
    """Returns (in_maps, perms, cap)."""
    tabf = np.zeros((TILES * P, D), dtype=np.float16)
    tabf[:VOCAB] = np.asarray(emb_table).astype(np.float16)
    tabf[0, :] = np.float16(1.0)

    cores = []
    cap = CAP_BAL
    for i in range(N_CORES):
        c = np.asarray(categories[i * B_LOC:(i + 1) * B_LOC]).reshape(N)
        c = c.astype(np.int64)
        counts = np.bincount(c, minlength=TILES * P)
        row2bin = _balance_bins(counts)
        if row2bin is None:                      # fallback: contiguous bins
            row2bin = np.arange(TILES * P) // P
            cap = max(cap, -(-int(np.bincount(
                c // P, minlength=TILES).max()) // P) * P)
        cores.append((c, row2bin))

    in_maps = []
    perms = []
    bpt = cap // P
    nblk = TILES * bpt
    npad = TILES * cap
    for c, row2bin in cores:
        # slot of each vocab row within its bin (stable order)
        row2slot = np.empty(TILES * P, np.int64)
        rows_of = []
        for b in range(TILES):
            rows = np.nonzero(row2bin == b)[0]
            row2slot[rows] = np.arange(len(rows))
            rows_of.append(rows)

        # per-core reordered table: tab_sb[k, b*D+f] = tabf[rows_of[b][k]]
        tab_sb = np.zeros((P, TILES * D), np.float16)
        for b in range(TILES):
            rows = rows_of[b]
            tab_sb[:len(rows), b * D:(b + 1) * D] = tabf[rows]

        bin_of_pos = row2bin[c]
        order = np.argsort(bin_of_pos, kind="stable")
        counts_b = np.bincount(bin_of_pos, minlength=TILES)
        assert counts_b.max() <= cap
        starts = np.zeros(TILES, np.int64)
        starts[1:] = np.cumsum(counts_b)[:-1]
        slot_of_sorted = (np.arange(N) - starts[bin_of_pos[order]]
                          + cap * bin_of_pos[order])
        perm = np.full(npad, -1, np.int64)
        perm[slot_of_sorted] = order

        x = np.asarray(inputs[len(perms) * B_LOC:
                              (len(perms) + 1) * B_LOC]).astype(
            np.float16).reshape(N, D)
        x_pad = np.zeros((npad, D), np.float16)
        crel = np.full(npad, 255, np.uint8)
        filled = perm >= 0
        x_pad[filled] = x[perm[filled]]
        crel[filled] = row2slot[c[perm[filled]]].astype(np.uint8)

        xbv = np.ascontiguousarray(
            x_pad.reshape(nblk, P, D).transpose(1, 0, 2)).reshape(P, nblk * D)
        crep = np.ascontiguousarray(
            np.broadcast_to(crel.reshape(1, npad), (P, npad)))
        in_maps.append({"xb": xbv, "catrel": crep, "tabsb": tab_sb})
        perms.append(perm)
    return in_maps, perms, cap


def kernel(inputs, categories, mask_positions=None, emb_table=None, **_):
    """Full (unsharded) inputs in, full output out. mask_positions unused."""
    in_maps, perms, cap = _shard_inputs(inputs, categories, emb_table)
    nc = _get_nc(cap)
    res = run_bass_kernel_spmd(nc, in_maps, list(range(N_CORES)))
    nblk = TILES * (cap // P)
    npad = TILES * cap
    out = np.empty((B, S, D), dtype=np.float32)
    for i in range(N_CORES):
        yv = res.results[i]["yb"].reshape(P, nblk, D).transpose(1, 0, 2)
        yv = yv.reshape(npad, D)
        perm = perms[i]
        filled = perm >= 0
        y = np.empty((N, D), np.float32)
        y[perm[filled]] = yv[filled].astype(np.float32)
        out[i * B_LOC:(i + 1) * B_LOC] = y.reshape(B_LOC, S, D)
    return out
